# revision 1
# baseline (speedup 1.0000x reference)
"""Multi-head attention block (B=8, N=1024, H=8, d=128, D_in=256) on 8 trn2 cores.

Sharding: data-parallel over batch — core b computes batch element b entirely
(8 heads), no collectives. Host pre-transposes x and B_bias and pre-scales wq
by 1/sqrt(d) so the device kernel needs no transposes or extra scaling.

Per-core dataflow (all matmuls float32r, moving free dim 512):
  QT[c,n], KT[c,n] = w.T @ x.T    (c-major so head slices are partition chunks)
  V[n,c]          = x @ wv        (n-major so PV stationary is a natural slice)
  per head h:
    S_T[m,n] = B_T[m,n] + KT_h[d,m].T @ QT_h[d,n]   (B added via identity-matmul
                                                     PSUM preload, q pre-scaled)
    attnT    = exp(S_T)            (no max subtraction: scores ~ N(0,1), safe)
    rowsum   = ones.T @ attnT      ([1,n] via matmul; softmax denominator)
    outT_h[d,n] = V_h[m,d].T @ attnT[m,n]            (unnormalized)
    oh = outT_h * bcast(1/rowsum)  (DMA partition-broadcast of the reciprocal)
    projT[j,n] += pw_h[c,j].T @ oh[c,n]              (accumulated over heads)
  yT = projT + proj_b  -> DRAM [128, 1024]; host transposes back.
"""

import math
import sys

import numpy as np

if "/opt/trn_rl_repo" not in sys.path:
    sys.path.insert(0, "/opt/trn_rl_repo")

import concourse.bass as bass
import concourse.tile as tile
from concourse import bacc
from concourse import mybir
from concourse.masks import make_identity

F32 = mybir.dt.float32
F32R = mybir.dt.float32r
EXP = mybir.ActivationFunctionType.Exp
IDENT = mybir.ActivationFunctionType.Identity

N = 1024          # sequence length
D_IN = 256        # input dim
H = 8             # heads
DH = 128          # head dim
C = H * DH        # 1024
NCORES = 8
HALF = 512        # matmul moving free dim


def r(ap):
    return ap  # tiles are allocated as float32r directly


def build_nc():
    nc = bacc.Bacc("TRN2", target_bir_lowering=False, debug=False,
                   num_devices=NCORES)

    xT = nc.dram_tensor("xT", [D_IN, N], F32R, kind="ExternalInput").ap()
    bT = nc.dram_tensor("bT", [N, N], F32R, kind="ExternalInput").ap()
    wq = nc.dram_tensor("wq", [D_IN, C], F32R, kind="ExternalInput").ap()
    wk = nc.dram_tensor("wk", [D_IN, C], F32R, kind="ExternalInput").ap()
    wv = nc.dram_tensor("wv", [D_IN, C], F32R, kind="ExternalInput").ap()
    wqb = nc.dram_tensor("wqb", [128, 8], F32, kind="ExternalInput").ap()
    wkb = nc.dram_tensor("wkb", [128, 8], F32, kind="ExternalInput").ap()
    wvbb = nc.dram_tensor("wvbb", [128, C], F32, kind="ExternalInput").ap()
    pw = nc.dram_tensor("pw", [C, DH], F32R, kind="ExternalInput").ap()
    pb = nc.dram_tensor("pb", [128, 1], F32, kind="ExternalInput").ap()
    yT = nc.dram_tensor("yT", [DH, N], F32, kind="ExternalOutput").ap()

    with tile.TileContext(nc) as tc:
        build_body(nc, tc, xT, bT, wq, wk, wv, wqb, wkb, wvbb, pw, pb, yT)
    nc.compile()
    return nc


def build_body(nc, tc, xT, bT, wq, wk, wv, wqb, wkb, wvbb, pw, pb, yT):
    with (
        tc.tile_pool(name="persist", bufs=1) as P,
        tc.tile_pool(name="attn", bufs=5) as AT,
        tc.tile_pool(name="outh", bufs=2) as OH,
        tc.tile_pool(name="rec", bufs=1) as RC,
        tc.tile_pool(name="dram", bufs=2, space="DRAM") as DR,
        tc.tile_pool(name="ps_s", bufs=2, space="PSUM") as PS_S,
        tc.tile_pool(name="ps_rs", bufs=2, space="PSUM") as PS_RS,
    ):
        # ---- persistent tiles ----
        # memset/affine_select don't take f32r dtypes; build in f32 and
        # convert via DVE copy (which rounds to f32r).
        ident = P.tile([128, 128], F32R, tag="ident")
        ones = P.tile([128, 1], F32R, tag="ones")
        with tc.tile_pool(name="mkconst", bufs=1) as MK:
            ident_f = MK.tile([128, 128], F32, tag="ident_f")
            make_identity(nc, ident_f)
            nc.vector.tensor_copy(ident, ident_f)
            ones_f = MK.tile([128, 1], F32, tag="ones_f")
            nc.vector.memset(ones_f, 1.0)
            nc.vector.tensor_copy(ones, ones_f)
        pb_sb = P.tile([128, 1], F32, tag="pb")
        nc.sync.dma_start(out=pb_sb, in_=pb)
        pw_sb = P.tile([128, 8, 128], F32R, tag="pw")
        nc.sync.dma_start(out=pw_sb, in_=pw.rearrange("(a p) j -> p a j", p=128))

        qt_sb = [P.tile([128, N], F32R, tag=f"qt{c}", name=f"qt{c}") for c in range(8)]
        kt_sb = [P.tile([128, N], F32R, tag=f"kt{c}", name=f"kt{c}") for c in range(8)]
        v_sb = [P.tile([128, C], F32R, tag=f"v{n}", name=f"v{n}") for n in range(8)]

        # ---- setup phase: load x/weights upfront, compute QT/KT/V ----
        with tc.tile_pool(name="setup", bufs=1) as S:
            xt_sb, wq_sb, wk_sb, wv_sb = [], [], [], []
            for d in range(2):
                t = S.tile([128, N], F32R, tag=f"xt{d}")
                nc.sync.dma_start(out=t, in_=xT[d * 128:(d + 1) * 128, :])
                xt_sb.append(t)
            for wname, wdram, lst in (("wq", wq, wq_sb), ("wk", wk, wk_sb),
                                      ("wv", wv, wv_sb)):
                for d in range(2):
                    t = S.tile([128, C], F32R, tag=f"{wname}{d}",
                               name=f"{wname}{d}")
                    nc.sync.dma_start(out=t, in_=wdram[d * 128:(d + 1) * 128, :])
                    lst.append(t)
            wqb_sb = S.tile([128, 8], F32, tag="wqb")
            nc.sync.dma_start(out=wqb_sb, in_=wqb)
            wkb_sb = S.tile([128, 8], F32, tag="wkb")
            nc.sync.dma_start(out=wkb_sb, in_=wkb)
            wvbb_sb = S.tile([128, C], F32, tag="wvbb")
            nc.sync.dma_start(out=wvbb_sb, in_=wvbb)

            # QT / KT: out[c128, n512] = w[d,c128].T @ xT[d, n512]
            for w_sb, b_sb, dst in ((wq_sb, wqb_sb, qt_sb),
                                    (wk_sb, wkb_sb, kt_sb)):
                for c in range(8):
                    cs = slice(c * 128, (c + 1) * 128)
                    for i in range(2):
                        ns = slice(i * HALF, (i + 1) * HALF)
                        ps = PS_S.tile([128, HALF], F32)
                        nc.tensor.matmul(ps, r(w_sb[0][:, cs]),
                                         r(xt_sb[0][:, ns]),
                                         start=True, stop=False)
                        nc.tensor.matmul(ps, r(w_sb[1][:, cs]),
                                         r(xt_sb[1][:, ns]),
                                         start=False, stop=True)
                        nc.scalar.activation(dst[c][:, ns], ps, func=IDENT,
                                             bias=b_sb[:, c:c + 1])

            # V: out[n128, c512] = xT[d, n128].T @ wv[d, c512], bias on DVE
            for n in range(8):
                nsl = slice(n * 128, (n + 1) * 128)
                for i in range(2):
                    cs = slice(i * HALF, (i + 1) * HALF)
                    ps = PS_S.tile([128, HALF], F32)
                    nc.tensor.matmul(ps, r(xt_sb[0][:, nsl]),
                                     r(wv_sb[0][:, cs]),
                                     start=True, stop=False)
                    nc.tensor.matmul(ps, r(xt_sb[1][:, nsl]),
                                     r(wv_sb[1][:, cs]),
                                     start=False, stop=True)
                    nc.vector.tensor_add(v_sb[n][:, cs], ps, wvbb_sb[:, cs])

        # ---- B_T loads (after weight loads so they don't hog DMA early) ----
        bt_sb = []
        for m in range(8):
            t = P.tile([128, N], F32R, tag=f"bt{m}", name=f"btl{m}")
            nc.sync.dma_start(out=t, in_=bT[m * 128:(m + 1) * 128, :])
            bt_sb.append(t)

        # ---- head loop, software-pipelined ----
        # PE stream per chunk t: S-group(t+1) is emitted before ones/PV(t) so
        # the PE never waits on ACT's exp; the per-head tail (recip -> DRAM
        # roundtrip bcast -> norm-mul) runs on DVE/DMA while the PE continues
        # into the next head; the proj matmuls are deferred 3 chunks so their
        # oh dependency is ready when the PE reaches them.
        yacc = P.tile([128, N], F32, tag="yacc")
        yt_sb = P.tile([128, N], F32, tag="yt")
        pv_t, rs_t, at_t = {}, {}, {}
        deferred = {}

        def s_group(h, m):
            ms = slice(m * 128, (m + 1) * 128)
            for i in range(2):
                ns = slice(i * HALF, (i + 1) * HALF)
                ps = PS_S.tile([128, HALF], F32)
                nc.tensor.matmul(ps, r(ident), r(bt_sb[m][:, ns]),
                                 start=True, stop=False)
                nc.tensor.matmul(ps, r(kt_sb[h][:, ms]), r(qt_sb[h][:, ns]),
                                 start=False, stop=True)
                at = AT.tile([128, HALF], F32R)
                nc.scalar.activation(at, ps, func=EXP)
                at_t[(h, m, i)] = at

        def ones_pv(h, m):
            hs = slice(h * 128, (h + 1) * 128)
            if m == 0:
                pv_t[h] = [PVP.tile([128, HALF], F32, tag="pvpj", name=f"pv{h}_{i}")
                           for i in range(2)]
                rs_t[h] = [PS_RS.tile([1, HALF], F32, tag="rs", name=f"rs{h}_{i}")
                           for i in range(2)]
            for i in range(2):
                at = at_t.pop((h, m, i))
                nc.tensor.matmul(rs_t[h][i], r(ones), r(at),
                                 start=(m == 0), stop=(m == 7))
                nc.tensor.matmul(pv_t[h][i], r(v_sb[m][:, hs]), r(at),
                                 start=(m == 0), stop=(m == 7))

        def head_tail(h):
            # off-PE: softmax denominators + partition-broadcast + normalize
            recip = RC.tile([1, N], F32, tag="recip", name=f"recip{h}")
            nc.vector.reciprocal(recip[:, 0:HALF], rs_t[h][0])
            nc.vector.reciprocal(recip[:, HALF:N], rs_t[h][1])
            scratch = DR.tile([N], F32, name=f"scr{h}")
            nc.sync.dma_start(out=scratch, in_=recip)
            bc = RC.tile([128, N], F32, tag="bc", name=f"bc{h}")
            nc.sync.dma_start(out=bc, in_=scratch.partition_broadcast(128))
            oh = OH.tile([128, N], F32R, tag="oh", name=f"oh{h}")
            for i in range(2):
                ns = slice(i * HALF, (i + 1) * HALF)
                nc.vector.tensor_mul(oh[:, ns], pv_t[h][i], bc[:, ns])
            return oh

        def proj_mms(h, oh):
            for i in range(2):
                ns = slice(i * HALF, (i + 1) * HALF)
                pj = PVP.tile([128, HALF], F32, tag="pvpj", name=f"pj{h}_{i}")
                nc.tensor.matmul(pj, r(pw_sb[:, h, :]), r(oh[:, ns]),
                                 start=True, stop=True)
                if h == 0:
                    nc.vector.tensor_copy(yacc[:, ns], pj)
                else:
                    nc.vector.tensor_add(yacc[:, ns], yacc[:, ns], pj)

        T = 64
        with tc.tile_pool(name="ps_pvpj", bufs=4, space="PSUM") as PVP:
            for t in range(T + 8):
                for cb in deferred.pop(t, ()):
                    cb()
                if t < T:
                    s_group(*divmod(t, 8))
                u = t - 1
                if 0 <= u < T:
                    h, m = divmod(u, 8)
                    ones_pv(h, m)
                    if m == 7:
                        oh = head_tail(h)
                        deferred.setdefault(t + 7, []).append(
                            lambda h=h, oh=oh: proj_mms(h, oh))

        for i in range(2):
            ns = slice(i * HALF, (i + 1) * HALF)
            nc.scalar.activation(yt_sb[:, ns], yacc[:, ns], func=IDENT,
                                 bias=pb_sb)
        nc.sync.dma_start(out=yT, in_=yt_sb)


_CACHE = {}


def _prep_inputs(x, B_bias, wq_w, wq_b, wk_w, wk_b, wv_w, wv_b, proj_w, proj_b):
    s = 1.0 / math.sqrt(DH)
    f = np.float32
    xTh = np.ascontiguousarray(x.transpose(0, 2, 1)).astype(f)      # [8,256,1024]
    bTh = np.ascontiguousarray(np.asarray(B_bias).T).astype(f)
    wq_s = (np.asarray(wq_w) * s).astype(f)
    wqb_t = np.ascontiguousarray((np.asarray(wq_b) * s).reshape(8, 128).T)
    wkb_t = np.ascontiguousarray(np.asarray(wk_b, f).reshape(8, 128).T)
    wvbb = np.ascontiguousarray(np.broadcast_to(np.asarray(wv_b, f), (128, C)))
    pb_t = np.ascontiguousarray(np.asarray(proj_b, f).reshape(128, 1))
    shared = dict(bT=bTh, wq=wq_s, wk=np.asarray(wk_w, f),
                  wv=np.asarray(wv_w, f), wqb=wqb_t, wkb=wkb_t, wvbb=wvbb,
                  pw=np.asarray(proj_w, f), pb=pb_t)
    return [dict(shared, xT=xTh[b]) for b in range(NCORES)]


def kernel(**inputs):
    from concourse.bass_utils import run_bass_kernel_spmd

    if "nc" not in _CACHE:
        _CACHE["nc"] = build_nc()
    nc = _CACHE["nc"]
    in_maps = _prep_inputs(**inputs)
    res = run_bass_kernel_spmd(nc, in_maps, core_ids=list(range(NCORES)))
    out = np.stack([np.asarray(res.results[b]["yT"]).T for b in range(NCORES)])
    return np.ascontiguousarray(out.astype(np.float32))



# revision 7
# speedup vs baseline: 1.1671x; 1.1671x over previous
"""Multi-head attention block (B=8, N=1024, H=8, d=128, D_in=256) on 8 trn2 cores.

Sharding: data-parallel over batch — core b computes batch element b entirely
(8 heads), no collectives. Host pre-transposes x and pre-scales wq by
1/sqrt(d); the additive [N,N] bias is shipped as exp(B)^T so the device does
exp(S+B) = exp(S) * expB with element-wise engines instead of an
identity-matmul PSUM preload (saves 65k PE rows).

Per-core dataflow (all matmuls float32r, moving free dim 512):
  QT[c,n], KT[c,n] = w.T @ x.T    (c-major so head slices are partition chunks)
  V[n,c]          = x @ wv        (n-major so PV stationary is a natural slice)
  head loop over 16 blocks t=(h,half), software-pipelined at m-granularity:
    S_T[m,n] = KT_h[d,m].T @ QT_h[d,n]     (single matmul per tile)
    at       = exp(S_T) * expB_T[m,n]      (ACT exp; mul split GPSIMD/DVE)
    rs[1,n]  = ones.T @ at                 (softmax denominator, PSUM-accum)
    pv[d,n]  = V_h[m,d].T @ at             (unnormalized, PSUM-accum)
    drain: recip -> DRAM-roundtrip partition-broadcast -> oh = pv * bc
    pj[j,n]  = pw_h[c,j].T @ oh ; yacc += pj
  yT = yacc + proj_b -> DRAM [128, 1024]; host transposes back.

Blocks are half-heads so each drain (DMA-latency-bound) hides behind the next
block's PE work. Emission order per block t: oh-mul(t-2) first (DVE), then the
m-loop [S(t,m) + ones/pv(t-1,m)], then recip/bcast(t-1), then pj/yacc(t-2) —
so the PE never queues behind a DMA-latency-bound op. PSUM: S/pj pool 3,
PV 3, RS 2 banks. QKV-projection setup is woven into blocks 0-7.
"""

import math
import sys

import numpy as np

if "/opt/trn_rl_repo" not in sys.path:
    sys.path.insert(0, "/opt/trn_rl_repo")

import ml_dtypes
import concourse.bass as bass
import concourse.tile as tile
from concourse import bacc
from concourse import mybir

F32 = mybir.dt.float32
F32R = mybir.dt.float32r
BF16 = mybir.dt.bfloat16
EXP = mybir.ActivationFunctionType.Exp
IDENT = mybir.ActivationFunctionType.Identity

N = 1024          # sequence length
D_IN = 256        # input dim
H = 8             # heads
DH = 128          # head dim
C = H * DH        # 1024
NCORES = 8
HALF = 512        # matmul moving free dim
NBLK = 16         # (head, half) blocks
POOL_MULS = 4     # expB muls per block routed to gpsimd (rest on DVE)


def build_nc():
    nc = bacc.Bacc("TRN2", target_bir_lowering=False, debug=False,
                   num_devices=NCORES)

    xT = nc.dram_tensor("xT", [D_IN, N], F32R, kind="ExternalInput").ap()
    eb = nc.dram_tensor("eb", [N, N], BF16, kind="ExternalInput").ap()
    wq = nc.dram_tensor("wq", [D_IN, C], F32R, kind="ExternalInput").ap()
    wk = nc.dram_tensor("wk", [D_IN, C], F32R, kind="ExternalInput").ap()
    wv = nc.dram_tensor("wv", [D_IN, C], F32R, kind="ExternalInput").ap()
    wqb = nc.dram_tensor("wqb", [128, 8], F32, kind="ExternalInput").ap()
    wkb = nc.dram_tensor("wkb", [128, 8], F32, kind="ExternalInput").ap()
    wvbb = nc.dram_tensor("wvbb", [128, C], F32, kind="ExternalInput").ap()
    pw = nc.dram_tensor("pw", [C, DH], F32R, kind="ExternalInput").ap()
    pb = nc.dram_tensor("pb", [128, 1], F32, kind="ExternalInput").ap()
    yT = nc.dram_tensor("yT", [DH, N], F32, kind="ExternalOutput").ap()

    with tile.TileContext(nc) as tc:
        build_body(nc, tc, xT, eb, wq, wk, wv, wqb, wkb, wvbb, pw, pb, yT)
    nc.compile()
    return nc


def build_body(nc, tc, xT, eb, wq, wk, wv, wqb, wkb, wvbb, pw, pb, yT):
    with (
        tc.tile_pool(name="persist", bufs=1) as P,
        tc.tile_pool(name="at", bufs=11) as AT,
        tc.tile_pool(name="oh", bufs=3) as OH,
        tc.tile_pool(name="bc", bufs=3) as BC,
        tc.tile_pool(name="rc", bufs=3) as RC,
        tc.tile_pool(name="dram", bufs=3, space="DRAM") as DR,
        tc.tile_pool(name="ps_s", bufs=3, space="PSUM") as PS_S,
        tc.tile_pool(name="ps_pj", bufs=1, space="PSUM") as PS_PJ,
        tc.tile_pool(name="ps_pv", bufs=3, space="PSUM") as PS_PV,
        tc.tile_pool(name="ps_rs", bufs=1, space="PSUM") as PS_RS,
    ):
        # ---- input DMAs, bandwidth-priority order ----
        xt2 = P.tile([128, 2, N], F32R, tag="xt2")
        nc.sync.dma_start(out=xt2, in_=xT.rearrange("(a p) n -> p a n", p=128))
        w2 = {}
        for wname, wdram in (("wq", wq), ("wk", wk), ("wv", wv)):
            t = P.tile([128, 2, C], F32R, tag=wname, name=wname)
            nc.sync.dma_start(out=t,
                              in_=wdram.rearrange("(a p) c -> p a c", p=128))
            w2[wname] = t
        eb_sb = []
        for m in range(8):
            t = P.tile([128, N], BF16, tag=f"eb{m}", name=f"ebl{m}")
            nc.sync.dma_start(out=t, in_=eb[m * 128:(m + 1) * 128, :])
            eb_sb.append(t)
        wqb_sb = P.tile([128, 8], F32, tag="wqb")
        nc.sync.dma_start(out=wqb_sb, in_=wqb)
        wkb_sb = P.tile([128, 8], F32, tag="wkb")
        nc.sync.dma_start(out=wkb_sb, in_=wkb)
        wvbb_sb = P.tile([128, C], F32, tag="wvbb")
        nc.sync.dma_start(out=wvbb_sb, in_=wvbb)
        pw_sb = P.tile([128, 8, 128], F32R, tag="pw")
        nc.sync.dma_start(out=pw_sb, in_=pw.rearrange("(a p) j -> p a j", p=128))
        pb_sb = P.tile([128, 1], F32, tag="pb")
        nc.sync.dma_start(out=pb_sb, in_=pb)

        # ---- persistent tiles ----
        ones = P.tile([128, 1], F32R, tag="ones")
        with tc.tile_pool(name="mkconst", bufs=1) as MK:
            ones_f = MK.tile([128, 1], F32, tag="ones_f")
            nc.vector.memset(ones_f, 1.0)
            nc.vector.tensor_copy(ones, ones_f)
        qt_sb = [P.tile([128, N], F32R, tag=f"qt{c}", name=f"qt{c}") for c in range(8)]
        kt_sb = [P.tile([128, N], F32R, tag=f"kt{c}", name=f"kt{c}") for c in range(8)]
        v_sb = [P.tile([128, C], F32R, tag=f"v{n}", name=f"v{n}") for n in range(8)]
        yacc = P.tile([128, N], F32, tag="yacc")
        yt_sb = P.tile([128, N], F32, tag="yt")

        # ---- setup pieces (emitted interleaved into early blocks) ----
        def qkt_piece(wname, b_sb, dst, c, on_act):
            cs = slice(c * 128, (c + 1) * 128)
            for i in range(2):
                ns = slice(i * HALF, (i + 1) * HALF)
                ps = PS_S.tile([128, HALF], F32)
                nc.tensor.matmul(ps, w2[wname][:, 0, cs], xt2[:, 0, ns],
                                 start=True, stop=False)
                nc.tensor.matmul(ps, w2[wname][:, 1, cs], xt2[:, 1, ns],
                                 start=False, stop=True)
                if on_act:
                    nc.scalar.activation(dst[c][:, ns], ps, func=IDENT,
                                         bias=b_sb[:, c:c + 1])
                else:
                    nc.vector.tensor_scalar_add(dst[c][:, ns], ps,
                                                b_sb[:, c:c + 1])

        def v_piece(n):
            nsl = slice(n * 128, (n + 1) * 128)
            for i in range(2):
                cs = slice(i * HALF, (i + 1) * HALF)
                ps = PS_S.tile([128, HALF], F32)
                nc.tensor.matmul(ps, xt2[:, 0, nsl], w2["wv"][:, 0, cs],
                                 start=True, stop=False)
                nc.tensor.matmul(ps, xt2[:, 1, nsl], w2["wv"][:, 1, cs],
                                 start=False, stop=True)
                nc.vector.tensor_add(v_sb[n][:, cs], ps, wvbb_sb[:, cs])

        # qt/kt c0 first so block 0's S matmuls can start immediately
        qkt_piece("wq", wqb_sb, qt_sb, 0, True)
        qkt_piece("wk", wkb_sb, kt_sb, 0, False)
        # remaining pieces woven into blocks: V into block 0 (needed by the
        # first ones/pv in block 1), qt/kt chunk c before block 2c
        pieces = [lambda n=n: v_piece(n) for n in range(8)]
        for c in range(1, 8):
            pieces.append(lambda c=c: qkt_piece("wq", wqb_sb, qt_sb, c, True))
            pieces.append(lambda c=c: qkt_piece("wk", wkb_sb, kt_sb, c, False))
        piece_quota = {0: 8, 1: 2, 2: 2, 3: 2, 4: 2, 5: 2, 6: 2, 7: 2}

        # ---- pipelined block loop: block t = (head h, n-half i) ----
        at_t = {}     # (t, m) -> at tile
        pv_t = {}     # t -> pv psum tile
        rs_t = {}     # t -> rowsum psum tile
        bc_t = {}     # t -> broadcast recip tile
        oh_t = {}     # t -> normalized head-output tile

        def s_exp_mul(t, m):
            h, i = divmod(t, 2)
            ns = slice(i * HALF, (i + 1) * HALF)
            ms = slice(m * 128, (m + 1) * 128)
            ps = PS_S.tile([128, HALF], F32)
            nc.tensor.matmul(ps, kt_sb[h][:, ms], qt_sb[h][:, ns],
                             start=True, stop=True)
            at = AT.tile([128, HALF], F32R)
            nc.scalar.activation(at, ps, func=EXP)
            eng = nc.gpsimd if m < POOL_MULS else nc.vector
            eng.tensor_mul(at, at, eb_sb[m][:, ns])
            at_t[(t, m)] = at

        def ones_pv(t, m):
            h, _ = divmod(t, 2)
            hs = slice(h * 128, (h + 1) * 128)
            if m == 0:
                rs_t[t] = PS_RS.tile([1, HALF], F32, tag="rs", name=f"rs{t}")
                pv_t[t] = PS_PV.tile([128, HALF], F32, tag="pv", name=f"pv{t}")
            at = at_t.pop((t, m))
            nc.tensor.matmul(rs_t[t], ones, at, start=(m == 0), stop=(m == 7))
            nc.tensor.matmul(pv_t[t], v_sb[m][:, hs], at,
                             start=(m == 0), stop=(m == 7))

        def drain_start(t):
            # softmax denominators: reciprocal + partition-broadcast roundtrip
            rc = RC.tile([1, HALF], F32, tag="rc", name=f"rc{t}")
            nc.vector.reciprocal(rc, rs_t.pop(t))
            scratch = DR.tile([HALF], F32, name=f"scr{t}")
            nc.sync.dma_start(out=scratch, in_=rc)
            bc = BC.tile([128, HALF], F32, tag="bc", name=f"bc{t}")
            nc.sync.dma_start(out=bc, in_=scratch.partition_broadcast(128))
            bc_t[t] = bc

        def oh_mul(t):
            oh = OH.tile([128, HALF], F32R, tag="oh", name=f"oh{t}")
            nc.vector.tensor_mul(oh, pv_t.pop(t), bc_t.pop(t))
            oh_t[t] = oh

        def proj_acc(t):
            h, i = divmod(t, 2)
            ns = slice(i * HALF, (i + 1) * HALF)
            pj = PS_PJ.tile([128, HALF], F32, tag="pj", name=f"pj{t}")
            nc.tensor.matmul(pj, pw_sb[:, h, :], oh_t.pop(t),
                             start=True, stop=True)
            if h == 0:
                nc.vector.tensor_copy(yacc[:, ns], pj)
            else:
                nc.vector.tensor_add(yacc[:, ns], yacc[:, ns], pj)

        def finalize(i):
            ns = slice(i * HALF, (i + 1) * HALF)
            nc.scalar.activation(yt_sb[:, ns], yacc[:, ns], func=IDENT,
                                 bias=pb_sb)
            nc.sync.dma_start(out=yT[:, ns], in_=yt_sb[:, ns])

        pi = 0
        for t in range(NBLK + 2):
            if 2 <= t:
                oh_mul(t - 2)     # first in this block's DVE queue: frees pv
            quota = piece_quota.get(t, 0)
            for m in range(8):
                if t < NBLK:
                    s_exp_mul(t, m)
                if 1 <= t <= NBLK:
                    ones_pv(t - 1, m)
                if quota and m % (8 // quota) == (8 // quota) - 1:
                    pieces[pi](); pi += 1
            if 1 <= t <= NBLK:
                drain_start(t - 1)   # rs(t-1) just stopped; launch DMA chain
            if 2 <= t:
                proj_acc(t - 2)      # PE reaches this after the block's work
                if t - 2 >= NBLK - 2:
                    finalize((t - 2) % 2)
        assert pi == len(pieces)


_CACHE = {}


def _prep_inputs(x, B_bias, wq_w, wq_b, wk_w, wk_b, wv_w, wv_b, proj_w, proj_b):
    s = 1.0 / math.sqrt(DH)
    f = np.float32
    xTh = np.ascontiguousarray(x.transpose(0, 2, 1)).astype(f)      # [8,256,1024]
    ebh = np.exp(np.asarray(B_bias, np.float32).T).astype(ml_dtypes.bfloat16)
    wq_s = (np.asarray(wq_w) * s).astype(f)
    wqb_t = np.ascontiguousarray((np.asarray(wq_b) * s).reshape(8, 128).T)
    wkb_t = np.ascontiguousarray(np.asarray(wk_b, f).reshape(8, 128).T)
    wvbb = np.ascontiguousarray(np.broadcast_to(np.asarray(wv_b, f), (128, C)))
    pb_t = np.ascontiguousarray(np.asarray(proj_b, f).reshape(128, 1))
    shared = dict(eb=ebh, wq=wq_s, wk=np.asarray(wk_w, f),
                  wv=np.asarray(wv_w, f), wqb=wqb_t, wkb=wkb_t, wvbb=wvbb,
                  pw=np.asarray(proj_w, f), pb=pb_t)
    return [dict(shared, xT=xTh[b]) for b in range(NCORES)]


def kernel(**inputs):
    from concourse.bass_utils import run_bass_kernel_spmd

    if "nc" not in _CACHE:
        _CACHE["nc"] = build_nc()
    nc = _CACHE["nc"]
    in_maps = _prep_inputs(**inputs)
    res = run_bass_kernel_spmd(nc, in_maps, core_ids=list(range(NCORES)))
    out = np.stack([np.asarray(res.results[b]["yT"]).T for b in range(NCORES)])
    return np.ascontiguousarray(out.astype(np.float32))
